# revision 1
# baseline (speedup 1.0000x reference)
"""Block-diagonal linear (grouped GEMM) on 8 TRN2 NeuronCores.

out[b, g*512+n] = sum_k x[b, g*512+k] * blocks[g, k, n]

Sharding: group-parallel — core g computes x[:, g*512:(g+1)*512] @ blocks[g].
Per-core kernel: for each 128-token tile, PE-transpose the x tile (fp32,
exact), round to float32r in the PSUM->SBUF copy, then run the K=512
accumulation as 4 float32r matmuls (full-rate on the PE at N=512).
"""
import numpy as np

import concourse.bacc as bacc
import concourse.tile as tile
from concourse import masks, mybir
from concourse.bass_utils import run_bass_kernel_spmd

TOKENS = 8192
G = 8
M = 512  # per-block in-features
N = 512  # per-block out-features
P = 128
MT = TOKENS // P  # 64 token tiles
KT = M // P       # 4 contraction tiles
F32 = mybir.dt.float32
F32R = mybir.dt.float32r

_CACHE: dict = {}


def _body(tc, nc, x, w, out):
    with (
        tc.tile_pool(name="const", bufs=1) as constp,
        tc.tile_pool(name="wp", bufs=1) as wp,
        tc.tile_pool(name="xin", bufs=4) as xin,
        tc.tile_pool(name="xtr", bufs=4) as xtr,
        tc.tile_pool(name="outp", bufs=4) as outp,
        tc.tile_pool(name="psx", bufs=3, space="PSUM") as psx,
        tc.tile_pool(name="pso", bufs=3, space="PSUM") as pso,
    ):
        ident = constp.tile([P, P], F32)
        masks.make_identity(nc, ident[:])

        # weights: [512, 512] -> [128, kt, 512], rounded once to f32r
        w_f = wp.tile([P, KT, N], F32, tag="wf")
        nc.sync.dma_start(w_f[:], w.rearrange("(j p) n -> p j n", p=P))
        w_r = wp.tile([P, KT, N], F32R, tag="wr")
        nc.vector.tensor_copy(w_r[:], w_f[:])

        for mt in range(MT):
            x_t = xin.tile([P, M], F32, tag="x")
            nc.sync.dma_start(x_t[:], x[mt * P:(mt + 1) * P, :])

            # transpose x tile: 4 blocks into one PSUM bank
            ps_xT = psx.tile([P, M], F32, tag="psx")
            for j in range(KT):
                nc.tensor.matmul(
                    ps_xT[:, j * P:(j + 1) * P],
                    x_t[:, j * P:(j + 1) * P],
                    ident[:],
                    is_transpose=True,
                    start=(j == 0),
                    stop=(j == KT - 1),
                )
            xT_r = xtr.tile([P, M], F32R, tag="xT")
            nc.vector.tensor_copy(xT_r[:], ps_xT[:])

            ps_o = pso.tile([P, N], F32, tag="pso")
            for j in range(KT):
                nc.tensor.matmul(
                    ps_o[:],
                    xT_r[:, j * P:(j + 1) * P],
                    w_r[:, j, :],
                    start=(j == 0),
                    stop=(j == KT - 1),
                )
            o_t = outp.tile([P, N], F32, tag="o")
            nc.scalar.copy(o_t[:], ps_o[:])
            nc.sync.dma_start(out[mt * P:(mt + 1) * P, :], o_t[:])


def _build():
    nc = bacc.Bacc("TRN2", target_bir_lowering=False, debug=False, num_devices=G)
    x = nc.dram_tensor("x", [TOKENS, M], F32, kind="ExternalInput").ap()
    w = nc.dram_tensor("w", [M, N], F32, kind="ExternalInput").ap()
    out = nc.dram_tensor("out", [TOKENS, N], F32, kind="ExternalOutput").ap()
    with tile.TileContext(nc) as tc:
        _body(tc, nc, x, w, out)
    nc.compile()
    return nc


def _run(in_maps, **kwargs):
    if "nc" not in _CACHE:
        _CACHE["nc"] = _build()
    return run_bass_kernel_spmd(_CACHE["nc"], in_maps, list(range(G)), **kwargs)


def _in_maps(x, blocks):
    return [
        {
            "x": np.ascontiguousarray(x[:, g * M:(g + 1) * M], dtype=np.float32),
            "w": np.ascontiguousarray(blocks[g], dtype=np.float32),
        }
        for g in range(G)
    ]


def kernel(x, blocks):
    x = np.asarray(x)
    blocks = np.asarray(blocks)
    res = _run(_in_maps(x, blocks))
    return np.concatenate([res.results[g]["out"] for g in range(G)], axis=1)


# revision 3
# speedup vs baseline: 1.1729x; 1.1729x over previous
"""Block-diagonal linear (grouped GEMM) on 8 TRN2 NeuronCores.

out[b, g*512+n] = sum_k x[b, g*512+k] * blocks[g, k, n]

Sharding: group-parallel — core g computes x[:, g*512:(g+1)*512] @ blocks[g].
Per-core kernel: for each 128-token tile, PE-transpose the x tile (fp32,
exact), round to float32r in the PSUM->SBUF copy, then run the K=512
accumulation as 4 float32r matmuls (full-rate on the PE at N=512).
"""
import numpy as np

import concourse.bacc as bacc
import concourse.tile as tile
from concourse import masks, mybir
from concourse.bass_utils import run_bass_kernel_spmd

TOKENS = 8192
G = 8
M = 512  # per-block in-features
N = 512  # per-block out-features
P = 128
MT = TOKENS // P  # 64 token tiles
KT = M // P       # 4 contraction tiles
F32 = mybir.dt.float32
F32R = mybir.dt.float32r

_CACHE: dict = {}


MB = 4  # m-tiles per DMA batch (1 MiB per transfer)


def _body(tc, nc, x, w, out):
    # DRAM views batched by MB m-tiles: [a, p, b, feat]
    x_v = x.rearrange("(a b p) k -> a p b k", b=MB, p=P)
    out_v = out.rearrange("(a b p) n -> a p b n", b=MB, p=P)
    with (
        tc.tile_pool(name="const", bufs=1) as constp,
        tc.tile_pool(name="wp", bufs=1) as wp,
        tc.tile_pool(name="xin", bufs=3) as xin,
        tc.tile_pool(name="xtr", bufs=4) as xtr,
        tc.tile_pool(name="outp", bufs=3) as outp,
        tc.tile_pool(name="psx", bufs=3, space="PSUM") as psx,
        tc.tile_pool(name="pso", bufs=3, space="PSUM") as pso,
    ):
        ident = constp.tile([P, P], F32)
        masks.make_identity(nc, ident[:])

        # weights: [512, 512] -> [128, kt, 512], rounded once to f32r
        w_f = wp.tile([P, KT, N], F32, tag="wf")
        nc.sync.dma_start(w_f[:], w.rearrange("(j p) n -> p j n", p=P))
        w_r = wp.tile([P, KT, N], F32R, tag="wr")
        nc.vector.tensor_copy(w_r[:], w_f[:])

        for a in range(MT // MB):
            x_t = xin.tile([P, MB, M], F32, tag="x")
            nc.sync.dma_start(x_t[:], x_v[a])
            o_t = outp.tile([P, MB, N], F32, tag="o")
            for b in range(MB):
                # transpose x tile: 4 blocks into one PSUM bank
                ps_xT = psx.tile([P, M], F32, tag="psx")
                for j in range(KT):
                    nc.tensor.matmul(
                        ps_xT[:, j * P:(j + 1) * P],
                        x_t[:, b, j * P:(j + 1) * P],
                        ident[:],
                        is_transpose=True,
                        start=(j == 0),
                        stop=(j == KT - 1),
                    )
                xT_r = xtr.tile([P, M], F32R, tag="xT")
                nc.vector.tensor_copy(xT_r[:], ps_xT[:])

                ps_o = pso.tile([P, N], F32, tag="pso")
                for j in range(KT):
                    nc.tensor.matmul(
                        ps_o[:],
                        xT_r[:, j * P:(j + 1) * P],
                        w_r[:, j, :],
                        start=(j == 0),
                        stop=(j == KT - 1),
                    )
                nc.scalar.copy(o_t[:, b, :], ps_o[:])
            nc.sync.dma_start(out_v[a], o_t[:])


def _build():
    nc = bacc.Bacc("TRN2", target_bir_lowering=False, debug=False, num_devices=G)
    x = nc.dram_tensor("x", [TOKENS, M], F32, kind="ExternalInput").ap()
    w = nc.dram_tensor("w", [M, N], F32, kind="ExternalInput").ap()
    out = nc.dram_tensor("out", [TOKENS, N], F32, kind="ExternalOutput").ap()
    with tile.TileContext(nc) as tc:
        _body(tc, nc, x, w, out)
    nc.compile()
    return nc


def _run(in_maps, **kwargs):
    if "nc" not in _CACHE:
        _CACHE["nc"] = _build()
    return run_bass_kernel_spmd(_CACHE["nc"], in_maps, list(range(G)), **kwargs)


def _in_maps(x, blocks):
    return [
        {
            "x": np.ascontiguousarray(x[:, g * M:(g + 1) * M], dtype=np.float32),
            "w": np.ascontiguousarray(blocks[g], dtype=np.float32),
        }
        for g in range(G)
    ]


def kernel(x, blocks):
    x = np.asarray(x)
    blocks = np.asarray(blocks)
    res = _run(_in_maps(x, blocks))
    return np.concatenate([res.results[g]["out"] for g in range(G)], axis=1)


# revision 4
# speedup vs baseline: 1.3905x; 1.1855x over previous
"""Block-diagonal linear (grouped GEMM) on 8 TRN2 NeuronCores.

out[b, g*512+n] = sum_k x[b, g*512+k] * blocks[g, k, n]

Sharding: group-parallel — core g computes x[:, g*512:(g+1)*512] @ blocks[g].
Per-core kernel: for each 128-token tile, PE-transpose the x tile (fp32,
exact), round to float32r in the PSUM->SBUF copy, then run the K=512
accumulation as 4 float32r matmuls (full-rate on the PE at N=512).
"""
import numpy as np

import concourse.bacc as bacc
import concourse.tile as tile
from concourse import masks, mybir
from concourse.bass_utils import run_bass_kernel_spmd

TOKENS = 8192
G = 8
M = 512  # per-block in-features
N = 512  # per-block out-features
P = 128
MT = TOKENS // P  # 64 token tiles
KT = M // P       # 4 contraction tiles
F32 = mybir.dt.float32
F32R = mybir.dt.float32r

_CACHE: dict = {}


def _batches():
    """Variable m-tile batch schedule: small head/tail for ramp, 4-tile steady."""
    sched = [1, 1, 2] + [4] * 14 + [2, 1, 1]
    assert sum(sched) == MT
    start = 0
    for n in sched:
        yield start, n
        start += n


def _body(tc, nc, x, w, out):
    x_v = x.rearrange("(m p) k -> m p k", p=P)      # [64, 128, 512]
    out_v = out.rearrange("(m p) n -> m p n", p=P)  # [64, 128, 512]
    with (
        tc.tile_pool(name="const", bufs=1) as constp,
        tc.tile_pool(name="wp", bufs=1) as wp,
        tc.tile_pool(name="xin", bufs=4) as xin,
        tc.tile_pool(name="xtr", bufs=6) as xtr,
        tc.tile_pool(name="outp", bufs=4) as outp,
        tc.tile_pool(name="psx", bufs=4, space="PSUM") as psx,
        tc.tile_pool(name="pso", bufs=4, space="PSUM") as pso,
    ):
        ident = constp.tile([P, P], F32)
        masks.make_identity(nc, ident[:])

        # weights: [512, 512] -> [128, kt, 512], rounded once to f32r
        w_f = wp.tile([P, KT, N], F32, tag="wf")
        nc.sync.dma_start(w_f[:], w.rearrange("(j p) n -> p j n", p=P))
        w_r = wp.tile([P, KT, N], F32R, tag="wr")
        nc.vector.tensor_copy(w_r[:], w_f[:])

        for m0, nb in _batches():
            x_t = xin.tile([P, 4, M], F32, tag="x")
            nc.sync.dma_start(
                x_t[:, :nb, :],
                x_v[m0:m0 + nb].rearrange("b p k -> p b k"),
            )
            o_t = outp.tile([P, 4, N], F32, tag="o")
            for b in range(nb):
                # transpose x tile: 4 blocks into one PSUM bank
                ps_xT = psx.tile([P, M], F32, tag="psx")
                for j in range(KT):
                    nc.tensor.matmul(
                        ps_xT[:, j * P:(j + 1) * P],
                        x_t[:, b, j * P:(j + 1) * P],
                        ident[:],
                        is_transpose=True,
                        start=(j == 0),
                        stop=(j == KT - 1),
                    )
                xT_r = xtr.tile([P, M], F32R, tag="xT")
                nc.vector.tensor_copy(xT_r[:], ps_xT[:])

                ps_o = pso.tile([P, N], F32, tag="pso")
                for j in range(KT):
                    nc.tensor.matmul(
                        ps_o[:],
                        xT_r[:, j * P:(j + 1) * P],
                        w_r[:, j, :],
                        start=(j == 0),
                        stop=(j == KT - 1),
                    )
                nc.scalar.copy(o_t[:, b, :], ps_o[:])
            nc.gpsimd.dma_start(
                out_v[m0:m0 + nb].rearrange("b p n -> p b n"),
                o_t[:, :nb, :],
            )


def _build():
    nc = bacc.Bacc("TRN2", target_bir_lowering=False, debug=False, num_devices=G)
    x = nc.dram_tensor("x", [TOKENS, M], F32, kind="ExternalInput").ap()
    w = nc.dram_tensor("w", [M, N], F32, kind="ExternalInput").ap()
    out = nc.dram_tensor("out", [TOKENS, N], F32, kind="ExternalOutput").ap()
    with tile.TileContext(nc) as tc:
        _body(tc, nc, x, w, out)
    nc.compile()
    return nc


def _run(in_maps, **kwargs):
    if "nc" not in _CACHE:
        _CACHE["nc"] = _build()
    return run_bass_kernel_spmd(_CACHE["nc"], in_maps, list(range(G)), **kwargs)


def _in_maps(x, blocks):
    return [
        {
            "x": np.ascontiguousarray(x[:, g * M:(g + 1) * M], dtype=np.float32),
            "w": np.ascontiguousarray(blocks[g], dtype=np.float32),
        }
        for g in range(G)
    ]


def kernel(x, blocks):
    x = np.asarray(x)
    blocks = np.asarray(blocks)
    res = _run(_in_maps(x, blocks))
    return np.concatenate([res.results[g]["out"] for g in range(G)], axis=1)
